# revision 19
# baseline (speedup 1.0000x reference)
"""Trainium2 Bass kernel for nn_Encoder (2-layer batched GCN + graph readout).

Reference computation (per graph b):
    h1 = relu(adj @ (x @ W1) + b1)          [N, H]
    h2 = relu(adj @ (h1 @ W2) + b2)         [N, H]
    gh = concat([h1.sum(0), h2.sum(0)])     [2H]

Sharding: data-parallel over batch B=256 across 8 cores (32 graphs/core).

Device-side layout trick: everything is computed in "transposed activation"
space so that the tensor engine never needs an on-chip transpose.
  - Host feeds xT[b]   = x[b].T    [F, N]   (f on partitions)
  - Host feeds adjT[b] = adj[b].T  [M, N]   (m on partitions)
  - S1 = x@W1 native [m, h]:   lhsT = xT slices,  rhs = W1
  - T1T = (adj@S1).T [h, n]:   lhsT = S1 chunks,  rhs = adjT chunks
  - h1T = relu(T1T + b1)  (bias is per-partition; row-sum -> gh1 for free
    via the activation accum_out port)
  - S2 = h1@W2 native [m, h]:  lhsT = h1T slices, rhs = W2
  - T2T = (adj@S2).T [h, n]:   lhsT = S2 chunks,  rhs = adjT chunks
  - h2T = relu(T2T + b2) stored transposed; host transposes back.
"""

import sys

sys.path.insert(0, "/opt/trn_rl_repo")

import numpy as np

import concourse.bass as bass
import concourse.mybir as mybir
import concourse.tile as tile
from concourse import bacc
from concourse.bass_utils import run_bass_kernel_spmd

B, N, F, H = 256, 360, 360, 128
NCORES = 8
G = B // NCORES          # graphs per core
PC = 120                 # partition-chunk size for the 360 dims
NCHUNK = N // PC         # 3

# dtype config: "in" = x/adj/W on the wire and in matmuls, "mid" = S1/h1T/S2
DT_IN_NP, DT_IN = np.float16, mybir.dt.float16
DT_MID = mybir.dt.float16


def _build_bass():
    nc = bacc.Bacc("TRN2", target_bir_lowering=False, debug=False)

    # host pre-arranges so each per-graph load is ONE contiguous DMA:
    # xT[g]   : [PC, NCHUNK, N]  with (p, c, n) = x[g, n, c*PC+p]
    # adjT[g] : [PC, NCHUNK, N]  with (p, c, n) = adj[g, n, c*PC+p]
    xT_d = nc.dram_tensor("xT", [G, PC, NCHUNK, N], DT_IN, kind="ExternalInput").ap()
    adjT_d = nc.dram_tensor("adjT", [G, PC, NCHUNK, N], DT_IN, kind="ExternalInput").ap()
    W1_d = nc.dram_tensor("W1", [PC, NCHUNK, H], DT_IN, kind="ExternalInput").ap()
    W2_d = nc.dram_tensor("W2", [H, H], DT_IN, kind="ExternalInput").ap()
    b1_d = nc.dram_tensor("b1", [H, 1], mybir.dt.float32, kind="ExternalInput").ap()
    b2_d = nc.dram_tensor("b2", [H, 1], mybir.dt.float32, kind="ExternalInput").ap()

    h2T_d = nc.dram_tensor("h2T", [G, H, N], mybir.dt.float32, kind="ExternalOutput").ap()
    gh1_d = nc.dram_tensor("gh1", [H, G], mybir.dt.float32, kind="ExternalOutput").ap()
    gh2_d = nc.dram_tensor("gh2", [H, G], mybir.dt.float32, kind="ExternalOutput").ap()

    relu = mybir.ActivationFunctionType.Relu

    with tile.TileContext(nc) as tc:
        with (
            tc.tile_pool(name="consts", bufs=1) as consts,
            tc.tile_pool(name="inp", bufs=6) as inp,
            tc.tile_pool(name="acts", bufs=3) as acts,
            tc.tile_pool(name="outs", bufs=4) as outs,
            tc.tile_pool(name="ghb", bufs=1) as ghb,
            tc.tile_pool(name="ps_s", bufs=4, space="PSUM") as ps_s,
            tc.tile_pool(name="ps_t", bufs=4, space="PSUM") as ps_t,
        ):
            # --- constants, loaded once ---
            W1_t = consts.tile([PC, NCHUNK, H], DT_IN)
            nc.sync.dma_start(W1_t[:], W1_d[:])
            W2_t = consts.tile([H, H], DT_IN)
            nc.sync.dma_start(W2_t[:], W2_d[:])
            b1_t = consts.tile([H, 1], mybir.dt.float32)
            nc.sync.dma_start(b1_t[:], b1_d[:])
            b2_t = consts.tile([H, 1], mybir.dt.float32)
            nc.sync.dma_start(b2_t[:], b2_d[:])

            ghbuf1 = ghb.tile([H, G], mybir.dt.float32)
            ghbuf2 = ghb.tile([H, G], mybir.dt.float32)
            # per-chunk partial row-sums; reduced to ghbuf* once at the end
            ghacc1 = ghb.tile([H, G, NCHUNK], mybir.dt.float32)
            ghacc2 = ghb.tile([H, G, NCHUNK], mybir.dt.float32)

            for g in range(G):
                # --- load graph inputs (transposed layouts) ---
                xT_t = inp.tile([PC, NCHUNK, N], DT_IN, tag="xT")
                adjT_t = inp.tile([PC, NCHUNK, N], DT_IN, tag="adjT")
                nc.sync.dma_start(xT_t[:], xT_d[g])
                nc.sync.dma_start(adjT_t[:], adjT_d[g])

                # --- layer 1: S1[m,h] = x @ W1 ---
                # all 3 m-chunks accumulate into ONE psum bank ([120, 3*128]
                # f32 = 1536B <= 2KB), freeing banks for cross-graph overlap
                S1_t = acts.tile([PC, NCHUNK, H], DT_MID, tag="S1")
                ps = ps_s.tile([PC, NCHUNK, H], mybir.dt.float32, tag="ps")
                for mc in range(NCHUNK):
                    for fc in range(NCHUNK):
                        nc.tensor.matmul(
                            ps[:, mc, :],
                            lhsT=xT_t[:, fc, mc * PC:(mc + 1) * PC],
                            rhs=W1_t[:, fc, :],
                            start=(fc == 0),
                            stop=(fc == NCHUNK - 1),
                        )
                nc.vector.tensor_copy(S1_t[:], ps[:])

                # --- T1T[h,n] = (adj @ S1).T ; h1T = relu(+b1) ---
                pt1 = ps_t.tile([H, N], mybir.dt.float32, tag="pt")
                for mc in range(NCHUNK):
                    nc.tensor.matmul(
                        pt1[:],
                        lhsT=S1_t[:, mc, :],
                        rhs=adjT_t[:, mc, :],
                        start=(mc == 0),
                        stop=(mc == NCHUNK - 1),
                    )
                # chunked relu: S2's first matmul can start after chunk 0
                h1T_t = acts.tile([H, N], DT_MID, tag="h1T")
                for c in range(NCHUNK):
                    nc.scalar.activation(
                        h1T_t[:, c * PC:(c + 1) * PC],
                        pt1[:, c * PC:(c + 1) * PC],
                        relu, bias=b1_t[:],
                        accum_out=ghacc1[:, g, c:c + 1],
                    )

                # --- layer 2: S2[m,h] = h1 @ W2 ---
                S2_t = acts.tile([PC, NCHUNK, H], DT_MID, tag="S2")
                ps2 = ps_s.tile([PC, NCHUNK, H], mybir.dt.float32, tag="ps")
                for mc in range(NCHUNK):
                    nc.tensor.matmul(
                        ps2[:, mc, :],
                        lhsT=h1T_t[:, mc * PC:(mc + 1) * PC],
                        rhs=W2_t[:],
                        start=True,
                        stop=True,
                    )
                nc.vector.tensor_copy(S2_t[:], ps2[:])

                # --- T2T[h,n] = (adj @ S2).T ; h2T = relu(+b2) ---
                pt2 = ps_t.tile([H, N], mybir.dt.float32, tag="pt")
                for mc in range(NCHUNK):
                    nc.tensor.matmul(
                        pt2[:],
                        lhsT=S2_t[:, mc, :],
                        rhs=adjT_t[:, mc, :],
                        start=(mc == 0),
                        stop=(mc == NCHUNK - 1),
                    )
                h2T_t = outs.tile([H, N], mybir.dt.float32, tag="h2T")
                for c in range(NCHUNK):
                    nc.scalar.activation(
                        h2T_t[:, c * PC:(c + 1) * PC],
                        pt2[:, c * PC:(c + 1) * PC],
                        relu, bias=b2_t[:],
                        accum_out=ghacc2[:, g, c:c + 1],
                    )

                # stores on the ACT HWDGE ring so they can't head-of-line
                # block the next graph's loads on the SP ring
                nc.scalar.dma_start(h2T_d[g], h2T_t[:])

            nc.vector.tensor_reduce(
                ghbuf1[:], ghacc1[:], mybir.AxisListType.X, mybir.AluOpType.add
            )
            nc.vector.tensor_reduce(
                ghbuf2[:], ghacc2[:], mybir.AxisListType.X, mybir.AluOpType.add
            )
            nc.scalar.dma_start(gh1_d[:], ghbuf1[:])
            nc.scalar.dma_start(gh2_d[:], ghbuf2[:])

    if not nc.is_finalized():
        nc.finalize()
    return nc


def _ensure_ntff_hook():
    """Make `antenv.axon_hooks` importable (the agent image's antenv stub
    lacks it) so run_bass_kernel_spmd(trace=True) can capture NTFF profiles."""
    import importlib.util

    if "antenv.axon_hooks" in sys.modules:
        return
    try:
        import antenv.axon_hooks  # noqa: F401
        return
    except ImportError:
        pass
    path = "/opt/trn_rl_repo/antenv/axon_hooks.py"
    spec = importlib.util.spec_from_file_location("antenv.axon_hooks", path)
    mod = importlib.util.module_from_spec(spec)
    spec.loader.exec_module(mod)
    sys.modules["antenv.axon_hooks"] = mod
    import antenv

    antenv.axon_hooks = mod


_CACHE: dict = {}


def _get_nc():
    if "nc" not in _CACHE:
        _CACHE["nc"] = _build_bass()
    return _CACHE["nc"]


def kernel(x, adj, W1, b1, W2, b2, trace=False, trace_kwargs=None):
    nc = _get_nc()

    # [B,N,(c p)] -> [B, p, c, N]: graph-transposed + chunked for 1-DMA loads
    xT = np.ascontiguousarray(
        x.reshape(B, N, NCHUNK, PC).transpose(0, 3, 2, 1)
    ).astype(DT_IN_NP, copy=False)
    adjT = np.ascontiguousarray(
        adj.reshape(B, N, NCHUNK, PC).transpose(0, 3, 2, 1)
    ).astype(DT_IN_NP, copy=False)
    W1c = np.ascontiguousarray(
        W1.reshape(NCHUNK, PC, H).transpose(1, 0, 2)
    ).astype(DT_IN_NP, copy=False)
    W2c = np.ascontiguousarray(W2).astype(DT_IN_NP, copy=False)
    b1c = np.ascontiguousarray(b1).reshape(H, 1).astype(np.float32, copy=False)
    b2c = np.ascontiguousarray(b2).reshape(H, 1).astype(np.float32, copy=False)

    in_maps = []
    for c in range(NCORES):
        sl = slice(c * G, (c + 1) * G)
        in_maps.append({
            "xT": xT[sl], "adjT": adjT[sl],
            "W1": W1c, "W2": W2c, "b1": b1c, "b2": b2c,
        })

    kw = {}
    if trace:
        _ensure_ntff_hook()
        kw = {"trace": True, "trace_kwargs": trace_kwargs or {}}
    res = run_bass_kernel_spmd(nc, in_maps, core_ids=list(range(NCORES)), **kw)

    h2 = np.empty((B, N, H), np.float32)
    gh = np.empty((B, 2 * H), np.float32)
    for c in range(NCORES):
        r = res.results[c]
        sl = slice(c * G, (c + 1) * G)
        h2[sl] = r["h2T"].transpose(0, 2, 1)
        gh[sl, :H] = r["gh1"].T
        gh[sl, H:] = r["gh2"].T

    if trace:
        _CACHE["last_result"] = res
    return (h2, gh)


# revision 27
# speedup vs baseline: 1.6426x; 1.6426x over previous
"""Trainium2 Bass kernel for nn_Encoder (2-layer batched GCN + graph readout).

Reference computation (per graph b):
    h1 = relu(adj @ (x @ W1) + b1)          [N, H]
    h2 = relu(adj @ (h1 @ W2) + b2)         [N, H]
    gh = concat([h1.sum(0), h2.sum(0)])     [2H]

Sharding: data-parallel over batch B=256 across 8 cores (32 graphs/core).

Device-side layout trick: everything is computed in "transposed activation"
space so that the tensor engine never needs an on-chip transpose.
  - Host feeds xT[b]   = x[b].T    [F, N]   (f on partitions)
  - Host feeds adjT[b] = adj[b].T  [M, N]   (m on partitions)
  - S1 = x@W1 native [m, h]:   lhsT = xT slices,  rhs = W1
  - T1T = (adj@S1).T [h, n]:   lhsT = S1 chunks,  rhs = adjT chunks
  - h1T = relu(T1T + b1)  (bias is per-partition; row-sum -> gh1 for free
    via the activation accum_out port)
  - S2 = h1@W2 native [m, h]:  lhsT = h1T slices, rhs = W2
  - T2T = (adj@S2).T [h, n]:   lhsT = S2 chunks,  rhs = adjT chunks
  - h2T = relu(T2T + b2) stored transposed; host transposes back.
"""

import sys

sys.path.insert(0, "/opt/trn_rl_repo")

import numpy as np

import concourse.bass as bass
import concourse.mybir as mybir
import concourse.tile as tile
from concourse import bacc
from concourse.bass_utils import run_bass_kernel_spmd

B, N, F, H = 256, 360, 360, 128
NCORES = 8
G = B // NCORES          # graphs per core
PC = 120                 # partition-chunk size for the 360 dims
NCHUNK = N // PC         # 3

# dtype config: "in" = x/adj/W on the wire and in matmuls, "mid" = S1/h1T/S2
DT_IN_NP, DT_IN = np.float16, mybir.dt.float16
DT_MID = mybir.dt.float16
DT_OUT = mybir.dt.float16  # h2T on the wire; host upcasts to fp32


def _build_bass():
    nc = bacc.Bacc("TRN2", target_bir_lowering=False, debug=False)

    # host pre-arranges so each per-graph load is ONE contiguous DMA:
    # xT[g]   : [PC, NCHUNK, N]  with (p, c, n) = x[g, n, c*PC+p]
    # adjT[g] : [PC, NCHUNK, N]  with (p, c, n) = adj[g, n, c*PC+p]
    xT_d = nc.dram_tensor("xT", [G, PC, NCHUNK, N], DT_IN, kind="ExternalInput").ap()
    adjT_d = nc.dram_tensor("adjT", [G, PC, NCHUNK, N], DT_IN, kind="ExternalInput").ap()
    W1_d = nc.dram_tensor("W1", [PC, NCHUNK, H], DT_IN, kind="ExternalInput").ap()
    W2_d = nc.dram_tensor("W2", [H, H], DT_IN, kind="ExternalInput").ap()
    b1_d = nc.dram_tensor("b1", [H, 1], mybir.dt.float32, kind="ExternalInput").ap()
    b2_d = nc.dram_tensor("b2", [H, 1], mybir.dt.float32, kind="ExternalInput").ap()

    h2T_d = nc.dram_tensor("h2T", [G, H, N], DT_OUT, kind="ExternalOutput").ap()
    gh1_d = nc.dram_tensor("gh1", [H, G], mybir.dt.float32, kind="ExternalOutput").ap()
    gh2_d = nc.dram_tensor("gh2", [H, G], mybir.dt.float32, kind="ExternalOutput").ap()

    relu = mybir.ActivationFunctionType.Relu

    with tile.TileContext(nc) as tc:
        with (
            tc.tile_pool(name="consts", bufs=1) as consts,
            tc.tile_pool(name="inp", bufs=6) as inp,
            tc.tile_pool(name="acts", bufs=3) as acts,
            tc.tile_pool(name="outs", bufs=4) as outs,
            tc.tile_pool(name="ghb", bufs=1) as ghb,
            tc.tile_pool(name="ps_s", bufs=4, space="PSUM") as ps_s,
            tc.tile_pool(name="ps_t", bufs=4, space="PSUM") as ps_t,
        ):
            # --- constants, loaded once ---
            W1_t = consts.tile([PC, NCHUNK, H], DT_IN)
            nc.sync.dma_start(W1_t[:], W1_d[:])
            W2_t = consts.tile([H, H], DT_IN)
            nc.sync.dma_start(W2_t[:], W2_d[:])
            b1_t = consts.tile([H, 1], mybir.dt.float32)
            nc.sync.dma_start(b1_t[:], b1_d[:])
            b2_t = consts.tile([H, 1], mybir.dt.float32)
            nc.sync.dma_start(b2_t[:], b2_d[:])

            ghbuf1 = ghb.tile([H, G], mybir.dt.float32)
            ghbuf2 = ghb.tile([H, G], mybir.dt.float32)

            for g in range(G):
                # --- load graph inputs (transposed layouts) ---
                xT_t = inp.tile([PC, NCHUNK, N], DT_IN, tag="xT")
                adjT_t = inp.tile([PC, NCHUNK, N], DT_IN, tag="adjT")
                nc.sync.dma_start(xT_t[:], xT_d[g])
                nc.sync.dma_start(adjT_t[:], adjT_d[g])

                # --- layer 1: S1[m,h] = x @ W1 ---
                # all 3 m-chunks accumulate into ONE psum bank ([120, 3*128]
                # f32 = 1536B <= 2KB), freeing banks for cross-graph overlap
                S1_t = acts.tile([PC, NCHUNK, H], DT_MID, tag="S1")
                ps = ps_s.tile([PC, NCHUNK, H], mybir.dt.float32, tag="ps")
                for mc in range(NCHUNK):
                    for fc in range(NCHUNK):
                        nc.tensor.matmul(
                            ps[:, mc, :],
                            lhsT=xT_t[:, fc, mc * PC:(mc + 1) * PC],
                            rhs=W1_t[:, fc, :],
                            start=(fc == 0),
                            stop=(fc == NCHUNK - 1),
                        )
                nc.vector.tensor_copy(S1_t[:], ps[:])

                # --- T1T[h,n] = (adj @ S1).T ; h1T = relu(+b1) ---
                pt1 = ps_t.tile([H, N], mybir.dt.float32, tag="pt")
                for mc in range(NCHUNK):
                    nc.tensor.matmul(
                        pt1[:],
                        lhsT=S1_t[:, mc, :],
                        rhs=adjT_t[:, mc, :],
                        start=(mc == 0),
                        stop=(mc == NCHUNK - 1),
                    )
                # layer-1 relu+bias fused on DVE (keeps ACT free; no separate
                # accumulator-read instruction on the DVE path)
                h1T_t = acts.tile([H, N], DT_MID, tag="h1T")
                nc.vector.tensor_scalar(
                    h1T_t[:], pt1[:], b1_t[:], 0.0,
                    mybir.AluOpType.add, mybir.AluOpType.max,
                )
                nc.vector.tensor_reduce(
                    ghbuf1[:, g:g + 1], h1T_t[:],
                    mybir.AxisListType.X, mybir.AluOpType.add,
                )

                # --- layer 2: S2[m,h] = h1 @ W2 ---
                S2_t = acts.tile([PC, NCHUNK, H], DT_MID, tag="S2")
                ps2 = ps_s.tile([PC, NCHUNK, H], mybir.dt.float32, tag="ps")
                for mc in range(NCHUNK):
                    nc.tensor.matmul(
                        ps2[:, mc, :],
                        lhsT=h1T_t[:, mc * PC:(mc + 1) * PC],
                        rhs=W2_t[:],
                        start=True,
                        stop=True,
                    )
                nc.vector.tensor_copy(S2_t[:], ps2[:])

                # --- T2T[h,n] = (adj @ S2).T ; h2T = relu(+b2) ---
                pt2 = ps_t.tile([H, N], mybir.dt.float32, tag="pt")
                for mc in range(NCHUNK):
                    nc.tensor.matmul(
                        pt2[:],
                        lhsT=S2_t[:, mc, :],
                        rhs=adjT_t[:, mc, :],
                        start=(mc == 0),
                        stop=(mc == NCHUNK - 1),
                    )
                h2T_t = outs.tile([H, N], DT_OUT, tag="h2T")
                nc.scalar.activation(
                    h2T_t[:], pt2[:], relu, bias=b2_t[:],
                    accum_out=ghbuf2[:, g:g + 1],
                )

                # stores on the ACT HWDGE ring so they can't head-of-line
                # block the next graph's loads on the SP ring
                nc.scalar.dma_start(h2T_d[g], h2T_t[:])

            nc.scalar.dma_start(gh1_d[:], ghbuf1[:])
            nc.scalar.dma_start(gh2_d[:], ghbuf2[:])

    if not nc.is_finalized():
        nc.finalize()
    return nc


def _ensure_ntff_hook():
    """Make `antenv.axon_hooks` importable (the agent image's antenv stub
    lacks it) so run_bass_kernel_spmd(trace=True) can capture NTFF profiles."""
    import importlib.util

    if "antenv.axon_hooks" in sys.modules:
        return
    try:
        import antenv.axon_hooks  # noqa: F401
        return
    except ImportError:
        pass
    path = "/opt/trn_rl_repo/antenv/axon_hooks.py"
    spec = importlib.util.spec_from_file_location("antenv.axon_hooks", path)
    mod = importlib.util.module_from_spec(spec)
    spec.loader.exec_module(mod)
    sys.modules["antenv.axon_hooks"] = mod
    import antenv

    antenv.axon_hooks = mod


_CACHE: dict = {}


def _get_nc():
    if "nc" not in _CACHE:
        _CACHE["nc"] = _build_bass()
    return _CACHE["nc"]


def kernel(x, adj, W1, b1, W2, b2, trace=False, trace_kwargs=None):
    nc = _get_nc()

    # [B,N,(c p)] -> [B, p, c, N]: graph-transposed + chunked for 1-DMA loads
    xT = np.ascontiguousarray(
        x.reshape(B, N, NCHUNK, PC).transpose(0, 3, 2, 1)
    ).astype(DT_IN_NP, copy=False)
    adjT = np.ascontiguousarray(
        adj.reshape(B, N, NCHUNK, PC).transpose(0, 3, 2, 1)
    ).astype(DT_IN_NP, copy=False)
    W1c = np.ascontiguousarray(
        W1.reshape(NCHUNK, PC, H).transpose(1, 0, 2)
    ).astype(DT_IN_NP, copy=False)
    W2c = np.ascontiguousarray(W2).astype(DT_IN_NP, copy=False)
    b1c = np.ascontiguousarray(b1).reshape(H, 1).astype(np.float32, copy=False)
    b2c = np.ascontiguousarray(b2).reshape(H, 1).astype(np.float32, copy=False)

    in_maps = []
    for c in range(NCORES):
        sl = slice(c * G, (c + 1) * G)
        in_maps.append({
            "xT": xT[sl], "adjT": adjT[sl],
            "W1": W1c, "W2": W2c, "b1": b1c, "b2": b2c,
        })

    kw = {}
    if trace:
        _ensure_ntff_hook()
        kw = {"trace": True, "trace_kwargs": trace_kwargs or {}}
    res = run_bass_kernel_spmd(nc, in_maps, core_ids=list(range(NCORES)), **kw)

    h2 = np.empty((B, N, H), np.float32)
    gh = np.empty((B, 2 * H), np.float32)
    for c in range(NCORES):
        r = res.results[c]
        sl = slice(c * G, (c + 1) * G)
        h2[sl] = r["h2T"].transpose(0, 2, 1).astype(np.float32)
        gh[sl, :H] = r["gh1"].T
        gh[sl, H:] = r["gh2"].T

    if trace:
        _CACHE["last_result"] = res
    return (h2, gh)
